# revision 14
# baseline (speedup 1.0000x reference)
"""Trainium2 Bass kernel for a GPT-2 style transformer block.

Sharding across 8 NeuronCores: cores 0-3 handle batch 0, cores 4-7 batch 1.
Within each 4-core group: tensor-parallel attention (3 heads/core, each core
computes LN1 + its QKV shard for the full 2048 tokens -> no communication),
row-sharded c_proj partials, then ONE ReduceScatter over the token dim, after
which each core owns 512 tokens and runs the MLP token-parallel.

Math notes:
 - LN gamma/beta folded into the following matmul weights/biases on host.
 - Softmax without max-subtraction (scores bounded ~ +-4 for this input
   distribution); sum-of-exp obtained free via a ones-augmented V column.
 - Causal mask applied multiplicatively (binary bf16 mask) after exp.
 - All matmuls bf16 with fp32 PSUM accumulation; LN/softmax stats fp32.
"""
import os
import sys

for _p in ("/opt/trn_rl_repo", "/root/.axon_site/_ro/trn_rl_repo"):
    if os.path.isdir(_p) and _p not in sys.path:
        sys.path.insert(0, _p)

import numpy as np
import ml_dtypes

from contextlib import ExitStack

import concourse.bass as bass
import concourse.tile as tile
from concourse import bacc, mybir
from concourse import bass_utils
from concourse.masks import make_identity

F32 = mybir.dt.float32
BF16 = mybir.dt.bfloat16
AF = mybir.ActivationFunctionType
ALU = mybir.AluOpType

B, T, C = 2, 2048, 768
H, D = 12, 64
NCORES = 8
GROUPS = [[0, 1, 2, 3], [4, 5, 6, 7]]
HPC = 3            # heads per core
TS = T // 4        # 512: token slice per core (post-RS)
FF = 4 * C         # 3072
NT = T // 128      # 16 token chunks
NCC = C // 128     # 6 channel chunks
NQB = 4            # q blocks
QB = 512
NFC = FF // 128    # 24 hidden chunks
EPS = 1e-5
ATT_SCALE = 1.0 / 8.0   # 1/sqrt(64)
QKW = 512   # padded qk weight cols: [Q0 Q1 | K0 K1 | Q2 pad | K2 pad]

_BUILT = {}


def _body(ctx, nc, tc, io):
    x, xs, wqk, bqk, wv, bv, wcp, bcp, wfc, bfc, wmp, bmp, mask, out = io

    cons = ctx.enter_context(tc.tile_pool(name="cons", bufs=1))
    xpool = ctx.enter_context(tc.tile_pool(name="xpool", bufs=3))
    lnpool = ctx.enter_context(tc.tile_pool(name="lnpool", bufs=3))
    stpool = ctx.enter_context(tc.tile_pool(name="stpool", bufs=4))
    big2k = ctx.enter_context(tc.tile_pool(name="big2k", bufs=1))
    qktp = ctx.enter_context(tc.tile_pool(name="qktp", bufs=1))
    vpool = ctx.enter_context(tc.tile_pool(name="vpool", bufs=1))
    ptpool = ctx.enter_context(tc.tile_pool(name="ptpool", bufs=6))
    ytp = ctx.enter_context(tc.tile_pool(name="ytp", bufs=1))
    invp = ctx.enter_context(tc.tile_pool(name="invp", bufs=2))
    cpp = ctx.enter_context(tc.tile_pool(name="cpp", bufs=2))
    rsp = ctx.enter_context(tc.tile_pool(name="rsp", bufs=2))
    h1p = ctx.enter_context(tc.tile_pool(name="h1p", bufs=1))
    h2tp = ctx.enter_context(tc.tile_pool(name="h2tp", bufs=1))
    wfcp = ctx.enter_context(tc.tile_pool(name="wfcp", bufs=6))
    wmpp = ctx.enter_context(tc.tile_pool(name="wmpp", bufs=1))
    outp = ctx.enter_context(tc.tile_pool(name="outp", bufs=2))
    ps = ctx.enter_context(tc.tile_pool(name="ps", bufs=4, space="PSUM"))
    psyt = ctx.enter_context(tc.tile_pool(name="psyt", bufs=2, space="PSUM"))
    pstp = ctx.enter_context(tc.tile_pool(name="pstp", bufs=2, space="PSUM"))
    dram = ctx.enter_context(tc.tile_pool(name="dram", bufs=1, space="DRAM"))

    # ---- constants / weights preload ----
    ident = cons.tile([128, 128], BF16)
    make_identity(nc, ident)
    eps_sb = cons.tile([128, 1], F32)
    nc.vector.memset(eps_sb, EPS)
    mask_sb = cons.tile([128, 896], BF16)
    nc.sync.dma_start(out=mask_sb, in_=mask)

    wqk_sb = []
    wv_sb = []
    for j in range(NCC):
        wq_t = cons.tile([128, QKW], BF16, name=f"wqk{j}", tag=f"wqk{j}")
        nc.sync.dma_start(out=wq_t, in_=wqk[128 * j:128 * (j + 1), :])
        wqk_sb.append(wq_t)
        wv_t = cons.tile([128, HPC * D], BF16, name=f"wv{j}", tag=f"wv{j}")
        nc.sync.dma_start(out=wv_t, in_=wv[128 * j:128 * (j + 1), :])
        wv_sb.append(wv_t)
    wcp_sb = []
    for h in range(HPC):
        wcp_t = cons.tile([64, C], BF16, name=f"wcp{h}", tag=f"wcp{h}")
        nc.sync.dma_start(out=wcp_t, in_=wcp[64 * h:64 * (h + 1), :])
        wcp_sb.append(wcp_t)

    def _col_bias(name, src, n):
        t = cons.tile([128, n], F32, name=name, tag=name)
        nc.sync.dma_start(out=t, in_=src.rearrange("(g p) -> p g", p=128))
        return t

    bqk_sb = _col_bias("bqk_sb", bqk, QKW // 128)   # [128, 4]
    bfc_sb = _col_bias("bfc_sb", bfc, NFC)                  # [128, 24]

    def _bcast(name, src, n):
        t = cons.tile([128, n], F32, name=name, tag=name)
        bc = bass.AP(tensor=src.tensor, offset=src.offset,
                     ap=[[0, 128]] + list(src.ap))
        nc.sync.dma_start(out=t, in_=bc)
        return t

    bv_bc = _bcast("bv_bc", bv, HPC * D)
    bcp_bc = _bcast("bcp_bc", bcp, C)
    bmp_bc = _bcast("bmp_bc", bmp, C)

    # ---- phase 1+2: LN1 over all T, transpose to hT [c, t] (bf16) ----
    hT = [big2k.tile([128, T], BF16, name=f"ht{j}", tag=f"hg{j}") for j in range(NCC)]

    def layernorm_chunk(x_t, ln_t):
        stats = stpool.tile([128, 3, 6], F32)
        mv = stpool.tile([128, 2], F32)
        sd = stpool.tile([128, 1], F32)
        rstd = stpool.tile([128, 1], F32)
        xg = x_t.rearrange("p (n s) -> p n s", s=256)
        for sg in range(3):
            nc.vector.bn_stats(out=stats[:, sg, :], in_=xg[:, sg, :])
        nc.vector.bn_aggr(out=mv, in_=stats)
        nc.scalar.activation(out=sd, in_=mv[:, 1:2], func=AF.Sqrt, bias=eps_sb)
        nc.vector.reciprocal(out=rstd, in_=sd)
        nc.vector.tensor_scalar(out=ln_t, in0=x_t, scalar1=mv[:, 0:1],
                                scalar2=rstd, op0=ALU.subtract, op1=ALU.mult)

    for i in range(NT):
        x_t = xpool.tile([128, C], F32, name="x_t", tag="x_t")
        nc.sync.dma_start(out=x_t, in_=x[128 * i:128 * (i + 1), :])
        ln_t = lnpool.tile([128, C], BF16, name="ln_t", tag="ln_t")
        layernorm_chunk(x_t, ln_t)
        for j in range(NCC):
            tp = pstp.tile([128, 128], BF16, name="tp", tag="tp")
            nc.tensor.transpose(out=tp, in_=ln_t[:, 128 * j:128 * (j + 1)],
                                identity=ident)
            nc.vector.tensor_copy(out=hT[j][:, 128 * i:128 * (i + 1)], in_=tp)

    # ---- phase 3: QK^T [512, T] (padded layout) and V_aug [t, 3, 65] ----
    qkT = [qktp.tile([128, T], BF16, name=f"qkt{g}", tag=f"qkt{g}") for g in range(4)]
    for g in range(4):
        for n in range(NQB):
            acc = ps.tile([128, QB], F32, name="acc", tag="acc")
            for j in range(NCC):
                nc.tensor.matmul(out=acc, lhsT=wqk_sb[j][:, 128 * g:128 * (g + 1)],
                                 rhs=hT[j][:, QB * n:QB * (n + 1)],
                                 start=(j == 0), stop=(j == NCC - 1))
            nc.scalar.activation(out=qkT[g][:, QB * n:QB * (n + 1)], in_=acc,
                                 func=AF.Identity, bias=bqk_sb[:, g:g + 1])

    v_sb = []
    for i in range(NT):
        v_t = vpool.tile([128, HPC, D + 1], BF16, name=f"v{i}", tag=f"v{i}")
        nc.vector.memset(v_t[:, :, D:D + 1], 1.0)
        acc = ps.tile([128, QB], F32, name="acc", tag="acc")
        for j in range(NCC):
            nc.tensor.matmul(out=acc[:, :HPC * D], lhsT=hT[j][:, 128 * i:128 * (i + 1)],
                             rhs=wv_sb[j], start=(j == 0), stop=(j == NCC - 1))
        nc.vector.tensor_tensor(
            out=v_t[:, :, 0:D],
            in0=acc[:, :HPC * D].rearrange("p (h d) -> p h d", d=D),
            in1=bv_bc.rearrange("p (h d) -> p h d", d=D), op=ALU.add)
        v_sb.append(v_t)

    # head h: Q^T in group [0,0,2][h] at partition offset [0,64,0][h];
    # K^T in the following group at the SAME offset (matmul quadrant rule).
    def qT_slice(h, nq):
        g, off = (0 if h < 2 else 2), 64 * (h % 2)
        return qkT[g][off:off + 64, QB * nq:QB * (nq + 1)]

    def kT_slice(h, kc):
        g, off = (1 if h < 2 else 3), 64 * (h % 2)
        return qkT[g][off:off + 64, 128 * kc:128 * (kc + 1)]

    # ---- phase 4: attention ----
    yT_sc = [ytp.tile([64, T], BF16, name=f"ytsc{h}", tag=f"ytsc{h}")
             for h in range(HPC)]
    for nq in range(NQB):
        nk = 4 * (nq + 1)
        for h in range(HPC):
            yt = psyt.tile([D + 1, QB], F32, name="yt", tag="yt")
            for kc in range(nk):
                st = ps.tile([128, QB], F32, name="st", tag="acc")
                nc.tensor.matmul(out=st, lhsT=kT_slice(h, kc), rhs=qT_slice(h, nq),
                                 start=True, stop=True)
                pt = ptpool.tile([128, QB], BF16, name="pt", tag="pt")
                nc.scalar.activation(out=pt, in_=st, func=AF.Exp, scale=ATT_SCALE)
                j = kc - 4 * nq
                if j >= 0:
                    nc.vector.tensor_tensor(
                        out=pt, in0=pt,
                        in1=mask_sb[:, 384 - 128 * j:896 - 128 * j], op=ALU.mult)
                nc.tensor.matmul(out=yt, lhsT=v_sb[kc][:, h, :], rhs=pt,
                                 start=(kc == 0), stop=(kc == nk - 1))
            inv = invp.tile([1, QB], F32, name="inv", tag="inv")
            nc.vector.reciprocal(out=inv, in_=yt[D:D + 1, :])
            invb = invp.tile([64, QB], F32, name="invb", tag="invb")
            nc.gpsimd.partition_broadcast(invb, inv)
            nc.vector.tensor_tensor(out=yT_sc[h][:, QB * nq:QB * (nq + 1)],
                                    in0=yt[0:D, :], in1=invb, op=ALU.mult)

    # ---- phase 5: c_proj partials -> DRAM bounce ----
    rs_in = dram.tile([T, C], BF16)
    rs_out = dram.tile([TS, C], BF16)
    for i in range(NT):
        cp_t = cpp.tile([128, C], BF16, name="cp_t", tag="cp_t")
        for fr in range(2):
            acc = ps.tile([128, 384], F32, name="acc2", tag="acc")
            for h in range(HPC):
                wslc = wcp_sb[h][:, 384 * fr:384 * (fr + 1)]
                nc.tensor.matmul(out=acc, lhsT=yT_sc[h][:, 128 * i:128 * (i + 1)],
                                 rhs=wslc, start=(h == 0), stop=(h == HPC - 1))
            nc.scalar.activation(out=cp_t[:, 384 * fr:384 * (fr + 1)], in_=acc,
                                 func=AF.Copy)
        nc.sync.dma_start(out=rs_in[128 * i:128 * (i + 1), :], in_=cp_t)

    # ---- phase 6: ReduceScatter over the 4-core batch group ----
    nc.gpsimd.collective_compute(
        "ReduceScatter", ALU.add, replica_groups=GROUPS,
        ins=[rs_in.opt()], outs=[rs_out.opt()])

    # ---- phase 7: residual + LN2 + transpose ----
    h1 = [h1p.tile([128, C], F32, name=f"h1_{i}", tag=f"h1_{i}") for i in range(4)]
    h2T = [h2tp.tile([128, TS], BF16, name=f"h2t{j}", tag=f"h2t{j}")
           for j in range(NCC)]
    for i in range(4):
        rs_t = rsp.tile([128, C], BF16, name="rs_t", tag="rs_t")
        nc.sync.dma_start(out=rs_t, in_=rs_out[128 * i:128 * (i + 1), :])
        xs_t = xpool.tile([128, C], F32, name="x_t", tag="x_t")
        nc.sync.dma_start(out=xs_t, in_=xs[128 * i:128 * (i + 1), :])
        nc.vector.tensor_tensor(out=h1[i], in0=xs_t, in1=rs_t, op=ALU.add)
        nc.vector.tensor_tensor(out=h1[i], in0=h1[i], in1=bcp_bc, op=ALU.add)
        ln_t = lnpool.tile([128, C], BF16, name="ln_t", tag="ln_t")
        layernorm_chunk(h1[i], ln_t)
        for j in range(NCC):
            tp = pstp.tile([128, 128], BF16, name="tp", tag="tp")
            nc.tensor.transpose(out=tp, in_=ln_t[:, 128 * j:128 * (j + 1)],
                                identity=ident)
            nc.vector.tensor_copy(out=h2T[j][:, 128 * i:128 * (i + 1)], in_=tp)

    # ---- phase 8: MLP ----
    gl = [big2k.tile([128, T], BF16, name=f"gl{j}", tag=f"hg{j}") for j in range(NCC)]
    for fi in range(NFC):
        acc = ps.tile([128, TS], F32, name="accf", tag="acc")
        for j in range(NCC):
            wfc_t = wfcp.tile([128, 128], BF16, name="wfc_t", tag="wfc_t")
            nc.sync.dma_start(
                out=wfc_t, in_=wfc[128 * j:128 * (j + 1), 128 * fi:128 * (fi + 1)])
            nc.tensor.matmul(out=acc, lhsT=wfc_t, rhs=h2T[j],
                             start=(j == 0), stop=(j == NCC - 1))
        jj, m = fi // 4, fi % 4
        nc.scalar.activation(out=gl[jj][:, TS * m:TS * (m + 1)], in_=acc,
                             func=AF.Gelu, bias=bfc_sb[:, fi:fi + 1])

    wmp_sb = []
    for fi in range(NFC):
        wmp_t = wmpp.tile([128, C], BF16, name=f"wmp{fi}", tag=f"wmp{fi}")
        nc.sync.dma_start(out=wmp_t, in_=wmp[128 * fi:128 * (fi + 1), :])
        wmp_sb.append(wmp_t)

    for i in range(4):
        out_t = outp.tile([128, C], F32, name="out_t", tag="out_t")
        for cr in range(2):
            acc = ps.tile([128, 384], F32, name="accm", tag="acc")
            for fi in range(NFC):
                jj, m = fi // 4, fi % 4
                lhs = gl[jj][:, TS * m + 128 * i:TS * m + 128 * (i + 1)]
                nc.tensor.matmul(out=acc, lhsT=lhs,
                                 rhs=wmp_sb[fi][:, 384 * cr:384 * (cr + 1)],
                                 start=(fi == 0), stop=(fi == NFC - 1))
            sl = slice(384 * cr, 384 * (cr + 1))
            nc.vector.tensor_tensor(out=out_t[:, sl], in0=acc, in1=h1[i][:, sl],
                                    op=ALU.add)
            nc.vector.tensor_tensor(out=out_t[:, sl], in0=out_t[:, sl],
                                    in1=bmp_bc[:, sl], op=ALU.add)
        nc.sync.dma_start(out=out[128 * i:128 * (i + 1), :], in_=out_t)


def build():
    if "nc" in _BUILT:
        return _BUILT["nc"]
    nc = bacc.Bacc("TRN2", target_bir_lowering=False, debug=False,
                   num_devices=NCORES)

    def din(name, shape, dt):
        return nc.dram_tensor(name, shape, dt, kind="ExternalInput").ap()

    io = (
        din("x", [T, C], F32),
        din("xs", [TS, C], F32),
        din("wqk", [C, QKW], BF16),
        din("bqk", [QKW], F32),
        din("wv", [C, HPC * D], BF16),
        din("bv", [HPC * D], F32),
        din("wcp", [HPC * D, C], BF16),
        din("bcp", [C], F32),
        din("wfc", [C, FF], BF16),
        din("bfc", [FF], F32),
        din("wmp", [FF, C], BF16),
        din("bmp", [C], F32),
        din("mask", [128, 896], BF16),
        nc.dram_tensor("out", [TS, C], F32, kind="ExternalOutput").ap(),
    )
    with tile.TileContext(nc) as tc, ExitStack() as ctx:
        _body(ctx, nc, tc, io)
    nc.finalize()
    return nc


def make_in_maps(inputs):
    """Host-side sharding: full inputs dict -> per-core in_maps."""
    f32 = np.float32
    bf = ml_dtypes.bfloat16
    x = np.asarray(inputs["x"], f32)
    ln1_g = np.asarray(inputs["ln1_g"], f32)
    ln1_b = np.asarray(inputs["ln1_b"], f32)
    W_attn = np.asarray(inputs["W_attn"], f32)
    b_attn = np.asarray(inputs["b_attn"], f32)
    W_cproj = np.asarray(inputs["W_cproj"], f32)
    b_cproj = np.asarray(inputs["b_cproj"], f32)
    ln2_g = np.asarray(inputs["ln2_g"], f32)
    ln2_b = np.asarray(inputs["ln2_b"], f32)
    W_fc = np.asarray(inputs["W_fc"], f32)
    b_fc = np.asarray(inputs["b_fc"], f32)
    W_mproj = np.asarray(inputs["W_mproj"], f32)
    b_mproj = np.asarray(inputs["b_mproj"], f32)

    Wa = ln1_g[:, None] * W_attn
    ba = b_attn + ln1_b @ W_attn
    Wf = ln2_g[:, None] * W_fc
    bf_ = b_fc + ln2_b @ W_fc

    p = np.arange(128)[:, None]
    c = np.arange(896)[None, :]
    mask = (c >= p + 384).astype(bf)

    maps = []
    for core in range(NCORES):
        b, s = core // 4, core % 4
        q0 = 192 * s
        zpad = np.zeros((C, 64), f32)
        # [Q0 Q1 | K0 K1 | Q2 pad | K2 pad]
        wqk = np.concatenate([
            Wa[:, q0:q0 + 128], Wa[:, 768 + q0:768 + q0 + 128],
            Wa[:, q0 + 128:q0 + 192], zpad,
            Wa[:, 768 + q0 + 128:768 + q0 + 192], zpad], axis=1)
        bqk = np.concatenate([
            ba[q0:q0 + 128], ba[768 + q0:768 + q0 + 128],
            ba[q0 + 128:q0 + 192], np.zeros(64, f32),
            ba[768 + q0 + 128:768 + q0 + 192], np.zeros(64, f32)])
        maps.append({
            "x": np.ascontiguousarray(x[b]),
            "xs": np.ascontiguousarray(x[b, TS * s:TS * (s + 1)]),
            "wqk": np.ascontiguousarray(wqk.astype(bf)),
            "bqk": np.ascontiguousarray(bqk),
            "wv": np.ascontiguousarray(Wa[:, 1536 + q0:1536 + q0 + 192].astype(bf)),
            "bv": np.ascontiguousarray(ba[1536 + q0:1536 + q0 + 192]),
            "wcp": np.ascontiguousarray(W_cproj[q0:q0 + 192, :].astype(bf)),
            "bcp": b_cproj,
            "wfc": np.ascontiguousarray(Wf.astype(bf)),
            "bfc": bf_,
            "wmp": np.ascontiguousarray(W_mproj.astype(bf)),
            "bmp": b_mproj,
            "mask": mask,
        })
    return maps


def kernel(**inputs):
    nc = build()
    _BUILT["nc"] = nc
    maps = make_in_maps(inputs)
    res = bass_utils.run_bass_kernel_spmd(nc, maps, core_ids=list(range(NCORES)))
    out = np.empty((B, T, C), np.float32)
    for core in range(NCORES):
        b, s = core // 4, core % 4
        out[b, TS * s:TS * (s + 1)] = res.results[core]["out"]
    return out


# revision 40
# speedup vs baseline: 283.4907x; 283.4907x over previous
"""Trainium2 Bass kernel for a GPT-2 style transformer block.

Sharding across 8 NeuronCores: cores 0-3 handle batch 0, cores 4-7 batch 1.
Within each 4-core group: tensor-parallel attention (3 heads/core, each core
computes LN1 + its QKV shard for the full 2048 tokens -> no communication),
row-sharded c_proj partials, then ONE ReduceScatter over the token dim, after
which each core owns 512 tokens and runs the MLP token-parallel.

Math notes:
 - LN gamma/beta folded into the following matmul weights/biases on host.
 - Softmax without max-subtraction (scores bounded ~ +-4 for this input
   distribution); sum-of-exp obtained free via a ones-augmented V column.
 - Causal mask applied multiplicatively (binary bf16 mask) after exp.
 - All matmuls bf16 with fp32 PSUM accumulation; LN/softmax stats fp32.
"""
import os
import sys

for _p in ("/opt/trn_rl_repo", "/root/.axon_site/_ro/trn_rl_repo"):
    if os.path.isdir(_p) and _p not in sys.path:
        sys.path.insert(0, _p)

import numpy as np
import ml_dtypes

from contextlib import ExitStack

import concourse.bass as bass
import concourse.tile as tile
from concourse import bacc, mybir
from concourse import bass_utils
from concourse.masks import make_identity

F32 = mybir.dt.float32
BF16 = mybir.dt.bfloat16
AF = mybir.ActivationFunctionType
ALU = mybir.AluOpType

B, T, C = 2, 2048, 768
H, D = 12, 64
NCORES = 8
GROUPS = [[0, 1, 2, 3], [4, 5, 6, 7]]
HPC = 3            # heads per core
TS = T // 4        # 512: token slice per core (post-RS)
FF = 4 * C         # 3072
NT = T // 128      # 16 token chunks
NCC = C // 128     # 6 channel chunks
NQB = 4            # q blocks
QB = 512
NFC = FF // 128    # 24 hidden chunks
EPS = 1e-5
ATT_SCALE = 1.0 / 8.0   # 1/sqrt(64)
QKW = 512   # padded qk weight cols: [Q0 Q1 | K0 K1 | Q2 pad | K2 pad]

_BUILT = {}


class _Pools:
    def __init__(self, ctx, tc):
        e = ctx.enter_context
        self.cons = e(tc.tile_pool(name="cons", bufs=1))
        self.xpool = e(tc.tile_pool(name="xpool", bufs=3))
        self.lnpool = e(tc.tile_pool(name="lnpool", bufs=4))
        self.stpool = e(tc.tile_pool(name="stpool", bufs=8))
        self.big2k = e(tc.tile_pool(name="big2k", bufs=1))
        self.qktp = e(tc.tile_pool(name="qktp", bufs=1))
        self.vpool = e(tc.tile_pool(name="vpool", bufs=1))
        self.ptpool = e(tc.tile_pool(name="ptpool", bufs=8))
        self.ytp = e(tc.tile_pool(name="ytp", bufs=1))
        self.invp = e(tc.tile_pool(name="invp", bufs=2))
        self.cpp = e(tc.tile_pool(name="cpp", bufs=2))
        self.rsp = e(tc.tile_pool(name="rsp", bufs=2))
        self.h1p = e(tc.tile_pool(name="h1p", bufs=1))
        self.h2tp = e(tc.tile_pool(name="h2tp", bufs=1))
        self.wfcp = e(tc.tile_pool(name="wfcp", bufs=8))
        self.wmpp = e(tc.tile_pool(name="wmpp", bufs=1))
        self.outp = e(tc.tile_pool(name="outp", bufs=2))
        self.ps = e(tc.tile_pool(name="ps", bufs=4, space="PSUM"))
        self.psyt = e(tc.tile_pool(name="psyt", bufs=2, space="PSUM"))
        self.pstp = e(tc.tile_pool(name="pstp", bufs=2, space="PSUM"))
        self.dram = e(tc.tile_pool(name="dram", bufs=1, space="DRAM"))


def _body(pools, nc, tc, io, timing=False, skip_att=False, skip_mlp=False):
    x, xs, wqk, bqk, wv, bv, wcp, bcp, wfc, bfc, wmp, bmp, mask, out = io
    cons, xpool, lnpool, stpool = pools.cons, pools.xpool, pools.lnpool, pools.stpool
    big2k, qktp, vpool, ptpool = pools.big2k, pools.qktp, pools.vpool, pools.ptpool
    ytp, invp, cpp, rsp = pools.ytp, pools.invp, pools.cpp, pools.rsp
    h1p, h2tp, wfcp, wmpp = pools.h1p, pools.h2tp, pools.wfcp, pools.wmpp
    outp, ps, psyt, pstp = pools.outp, pools.ps, pools.psyt, pools.pstp
    dram = pools.dram

    # ---- constants ----
    ident = cons.tile([128, 128], BF16)
    make_identity(nc, ident)
    eps_sb = cons.tile([128, 1], F32)
    nc.vector.memset(eps_sb, EPS)

    # ---- x loads first (batched 2 chunks per DMA), LN1, transpose ----
    hT_big = big2k.tile([128, NCC, T], BF16, name="hT_big", tag="hg")

    def layernorm_chunk(x_t, ln_t):
        stats = stpool.tile([128, 3, 6], F32)
        mv = stpool.tile([128, 2], F32)
        sd = stpool.tile([128, 1], F32)
        rstd = stpool.tile([128, 1], F32)
        xg = x_t.rearrange("p (n s) -> p n s", s=256)
        for sg in range(3):
            nc.vector.bn_stats(out=stats[:, sg, :], in_=xg[:, sg, :])
        nc.vector.bn_aggr(out=mv, in_=stats)
        nc.scalar.activation(out=sd, in_=mv[:, 1:2], func=AF.Sqrt, bias=eps_sb)
        nc.vector.reciprocal(out=rstd, in_=sd)
        nc.gpsimd.tensor_scalar(out=ln_t, in0=x_t, scalar1=mv[:, 0:1],
                                scalar2=rstd, op0=ALU.subtract, op1=ALU.mult)

    def transpose_chunk(ln_t, dst_big, i):
        # 6 PE transposes into one PSUM row, one batched DVE copy out
        tpr = pstp.tile([128, NCC, 128], BF16, name="tpr", tag="tp")
        for j in range(NCC):
            nc.tensor.transpose(out=tpr[:, j, :], in_=ln_t[:, 128 * j:128 * (j + 1)],
                                identity=ident)
        nc.vector.tensor_copy(out=dst_big[:, :, 128 * i:128 * (i + 1)], in_=tpr)

    x_dma_last = None
    for q in range(NT // 2):
        xq = xpool.tile([128, 2, C], F32, name="xq", tag="xq")
        src = x[256 * q:256 * (q + 1), :].rearrange("(r p) c -> p r c", p=128)
        x_dma_last = nc.sync.dma_start(out=xq, in_=src)
        for r in range(2):
            i = 2 * q + r
            ln_t = lnpool.tile([128, C], BF16, name="ln_t", tag="ln_t")
            layernorm_chunk(xq[:, r, :], ln_t)
            transpose_chunk(ln_t, hT_big, i)
    hT = [hT_big[:, j, :] for j in range(NCC)]

    # ---- weights/bias loads (after x) ----
    mask_sb = cons.tile([128, 896], BF16)
    nc.sync.dma_start(out=mask_sb, in_=mask)

    wqk_sb = []
    wv_sb = []
    for j in range(NCC):
        wq_t = cons.tile([128, QKW], BF16, name=f"wqk{j}", tag=f"wqk{j}")
        nc.sync.dma_start(out=wq_t, in_=wqk[128 * j:128 * (j + 1), :])
        wqk_sb.append(wq_t)
        wv_t = cons.tile([128, HPC * D], BF16, name=f"wv{j}", tag=f"wv{j}")
        nc.sync.dma_start(out=wv_t, in_=wv[128 * j:128 * (j + 1), :])
        wv_sb.append(wv_t)
    wcp_sb = []
    for h in range(HPC):
        wcp_t = cons.tile([64, C], BF16, name=f"wcp{h}", tag=f"wcp{h}")
        nc.sync.dma_start(out=wcp_t, in_=wcp[64 * h:64 * (h + 1), :])
        wcp_sb.append(wcp_t)

    def _col_bias(name, src, n):
        t = cons.tile([128, n], F32, name=name, tag=name)
        nc.sync.dma_start(out=t, in_=src.rearrange("(g p) -> p g", p=128))
        return t

    bqk_sb = _col_bias("bqk_sb", bqk, QKW // 128)   # [128, 4]
    bfc_sb = _col_bias("bfc_sb", bfc, NFC)                  # [128, 24]

    def _bcast(name, src, n):
        t = cons.tile([128, n], F32, name=name, tag=name)
        bc = bass.AP(tensor=src.tensor, offset=src.offset,
                     ap=[[0, 128]] + list(src.ap))
        nc.sync.dma_start(out=t, in_=bc)
        return t

    bv_bc = _bcast("bv_bc", bv, HPC * D)
    bmp_bc = _bcast("bmp_bc", bmp, C)

    # ---- phase 3: QK^T [512, T] (padded layout) and V_aug [t, 3, 65] ----
    qkT = [qktp.tile([128, T], BF16, name=f"qkt{g}", tag=f"qkt{g}") for g in range(4)]
    for g in range(4):
        for n in range(NQB):
            acc = ps.tile([128, QB], F32, name="acc", tag="acc")
            for j in range(NCC):
                nc.tensor.matmul(out=acc, lhsT=wqk_sb[j][:, 128 * g:128 * (g + 1)],
                                 rhs=hT[j][:, QB * n:QB * (n + 1)],
                                 start=(j == 0), stop=(j == NCC - 1))
            nc.vector.tensor_scalar_add(out=qkT[g][:, QB * n:QB * (n + 1)],
                                        in0=acc, scalar1=bqk_sb[:, g:g + 1])

    v_sb = []
    for i in range(NT):
        v_t = vpool.tile([128, HPC, D + 1], BF16, name=f"v{i}", tag=f"v{i}")
        nc.vector.memset(v_t[:, :, D:D + 1], 1.0)
        acc = ps.tile([128, QB], F32, name="acc", tag="acc")
        for j in range(NCC):
            nc.tensor.matmul(out=acc[:, :HPC * D], lhsT=hT[j][:, 128 * i:128 * (i + 1)],
                             rhs=wv_sb[j], start=(j == 0), stop=(j == NCC - 1))
        nc.vector.tensor_tensor(
            out=v_t[:, :, 0:D],
            in0=acc[:, :HPC * D].rearrange("p (h d) -> p h d", d=D),
            in1=bv_bc.rearrange("p (h d) -> p h d", d=D), op=ALU.add)
        v_sb.append(v_t)

    # head h: Q^T in group [0,0,2][h] at partition offset [0,64,0][h];
    # K^T in the following group at the SAME offset (matmul quadrant rule).
    def qT_slice(h, nq):
        g, off = (0 if h < 2 else 2), 64 * (h % 2)
        return qkT[g][off:off + 64, QB * nq:QB * (nq + 1)]

    def kT_slice(h, kc):
        g, off = (1 if h < 2 else 3), 64 * (h % 2)
        return qkT[g][off:off + 64, 128 * kc:128 * (kc + 1)]

    # ---- phase 4: attention ----
    yT_sc = [ytp.tile([64, T], BF16, name=f"ytsc{h}", tag=f"ytsc{h}")
             for h in range(HPC)]
    if skip_att:
        for h in range(HPC):
            nc.vector.memset(yT_sc[h], 0.001)
    for nq in range(NQB if not skip_att else 0):
        nk = 4 * (nq + 1)
        for h in range(HPC):
            yt = psyt.tile([D + 1, QB], F32, name="yt", tag="yt")
            for kc in range(nk):
                j = kc - 4 * nq
                f0 = max(0, 128 * j)   # cols < f0 are fully masked
                st = ps.tile([128, QB], F32, name="st", tag="acc")
                nc.tensor.matmul(out=st[:, f0:], lhsT=kT_slice(h, kc),
                                 rhs=qT_slice(h, nq)[:, f0:],
                                 start=True, stop=True)
                pt = ptpool.tile([128, QB], BF16, name="pt", tag="pt")
                nc.scalar.activation(out=pt[:, f0:], in_=st[:, f0:],
                                     func=AF.Exp, scale=ATT_SCALE)
                if j >= 0:
                    nc.vector.tensor_tensor(
                        out=pt[:, f0:], in0=pt[:, f0:],
                        in1=mask_sb[:, 384:896 - f0], op=ALU.mult)
                nc.tensor.matmul(out=yt[:, f0:], lhsT=v_sb[kc][:, h, :],
                                 rhs=pt[:, f0:],
                                 start=(kc == 0), stop=(kc == nk - 1))
            inv = invp.tile([1, QB], F32, name="inv", tag="inv")
            nc.vector.reciprocal(out=inv, in_=yt[D:D + 1, :])
            invb = invp.tile([64, QB], F32, name="invb", tag="invb")
            nc.gpsimd.partition_broadcast(invb, inv)
            nc.vector.tensor_tensor(out=yT_sc[h][:, QB * nq:QB * (nq + 1)],
                                    in0=yt[0:D, :], in1=invb, op=ALU.mult)

    # ---- phase 5: c_proj partials -> DRAM bounce ----
    rs_in = dram.tile([T, C], BF16)
    rs_out = dram.tile([TS, C], BF16)
    for i in range(NT):
        cp_t = cpp.tile([128, C], BF16, name="cp_t", tag="cp_t")
        for fr in range(2):
            acc = ps.tile([128, 384], F32, name="acc2", tag="acc")
            for h in range(HPC):
                wslc = wcp_sb[h][:, 384 * fr:384 * (fr + 1)]
                nc.tensor.matmul(out=acc, lhsT=yT_sc[h][:, 128 * i:128 * (i + 1)],
                                 rhs=wslc, start=(h == 0), stop=(h == HPC - 1))
            nc.scalar.copy(out=cp_t[:, 384 * fr:384 * (fr + 1)], in_=acc)
        nc.sync.dma_start(out=rs_in[128 * i:128 * (i + 1), :], in_=cp_t)

    # ---- phase 6: ReduceScatter over the 4-core batch group ----
    if timing:
        # timing-only build (TimelineSim can't model collectives): stand-in DMA
        nc.sync.dma_start(out=rs_out, in_=rs_in[0:TS, :])
    else:
        nc.gpsimd.collective_compute(
            "ReduceScatter", ALU.add, replica_groups=GROUPS,
            ins=[rs_in.opt()], outs=[rs_out.opt()])

    # ---- phase 7: residual + LN2 + transpose ----
    h1 = [h1p.tile([128, C], F32, name=f"h1_{i}", tag=f"h1_{i}") for i in range(4)]
    h2T_big = h2tp.tile([128, NCC, TS], BF16, name="h2T_big", tag="h2t")
    for q in range(2):
        rs_q = rsp.tile([128, 2, C], BF16, name="rs_q", tag="rs_q")
        nc.sync.dma_start(
            out=rs_q,
            in_=rs_out[256 * q:256 * (q + 1), :].rearrange("(r p) c -> p r c", p=128))
        xs_q = xpool.tile([128, 2, C], F32, name="xq", tag="xq")
        nc.sync.dma_start(
            out=xs_q,
            in_=xs[256 * q:256 * (q + 1), :].rearrange("(r p) c -> p r c", p=128))
        for r in range(2):
            i = 2 * q + r
            nc.gpsimd.tensor_tensor(out=h1[i], in0=xs_q[:, r, :], in1=rs_q[:, r, :],
                                    op=ALU.add)
            ln_t = lnpool.tile([128, C], BF16, name="ln_t", tag="ln_t")
            layernorm_chunk(h1[i], ln_t)
            transpose_chunk(ln_t, h2T_big, i)
    h2T = [h2T_big[:, j, :] for j in range(NCC)]

    # ---- phase 8: MLP ----
    if skip_mlp:
        for i in range(4):
            out_t = outp.tile([128, C], F32, name="out_t", tag="out_t")
            nc.vector.tensor_copy(out=out_t, in_=h1[i])
            nc.sync.dma_start(out=out[128 * i:128 * (i + 1), :], in_=out_t)
        return
    # fc weights streamed as [128, 768] slabs (6 f-chunks per DMA): 24 DMAs
    gl_big = big2k.tile([128, NCC, T], BF16, name="gl_big", tag="hg")
    gl = [gl_big[:, j, :] for j in range(NCC)]
    for fg in range(4):
        slabs = []
        for j in range(NCC):
            wfc_t = wfcp.tile([128, 768], BF16, name="wfc_t", tag="wfc_t")
            d = nc.sync.dma_start(
                out=wfc_t, in_=wfc[128 * j:128 * (j + 1), 768 * fg:768 * (fg + 1)])
            tile.add_dep_helper(d.ins, x_dma_last.ins, sync=False,
                                reason="defer wfc prefetch past x load")
            slabs.append(wfc_t)
        for fl in range(6):
            fi = 6 * fg + fl
            acc = ps.tile([128, TS], F32, name="accf", tag="acc")
            for j in range(NCC):
                nc.tensor.matmul(out=acc, lhsT=slabs[j][:, 128 * fl:128 * (fl + 1)],
                                 rhs=h2T[j], start=(j == 0), stop=(j == NCC - 1))
            jj, m = fi // 4, fi % 4
            nc.scalar.activation(out=gl[jj][:, TS * m:TS * (m + 1)], in_=acc,
                                 func=AF.Gelu, bias=bfc_sb[:, fi:fi + 1])

    wmp_sb = []
    for fi in range(NFC):
        wmp_t = wmpp.tile([128, C], BF16, name=f"wmp{fi}", tag=f"wmp{fi}")
        d = nc.sync.dma_start(out=wmp_t, in_=wmp[128 * fi:128 * (fi + 1), :])
        tile.add_dep_helper(d.ins, x_dma_last.ins, sync=False,
                            reason="defer wmp prefetch past x load")
        wmp_sb.append(wmp_t)

    for i in range(4):
        out_t = outp.tile([128, C], F32, name="out_t", tag="out_t")
        for cr in range(2):
            acc = ps.tile([128, 384], F32, name="accm", tag="acc")
            for fi in range(NFC):
                jj, m = fi // 4, fi % 4
                lhs = gl[jj][:, TS * m + 128 * i:TS * m + 128 * (i + 1)]
                nc.tensor.matmul(out=acc, lhsT=lhs,
                                 rhs=wmp_sb[fi][:, 384 * cr:384 * (cr + 1)],
                                 start=(fi == 0), stop=(fi == NFC - 1))
            sl = slice(384 * cr, 384 * (cr + 1))
            nc.vector.tensor_tensor(out=out_t[:, sl], in0=acc, in1=h1[i][:, sl],
                                    op=ALU.add)
            nc.vector.tensor_tensor(out=out_t[:, sl], in0=out_t[:, sl],
                                    in1=bmp_bc[:, sl], op=ALU.add)
        nc.sync.dma_start(out=out[128 * i:128 * (i + 1), :], in_=out_t)


def build(timing=False, loop_n=1, skip_att=False, skip_mlp=False):
    key = ("nc", timing, loop_n, skip_att, skip_mlp)
    if key in _BUILT:
        return _BUILT[key]
    nc = bacc.Bacc("TRN2", target_bir_lowering=False, debug=False,
                   num_devices=1 if timing else NCORES)

    def din(name, shape, dt):
        return nc.dram_tensor(name, shape, dt, kind="ExternalInput").ap()

    io = (
        din("x", [T, C], F32),
        din("xs", [TS, C], F32),
        din("wqk", [C, QKW], BF16),
        din("bqk", [QKW], F32),
        din("wv", [C, HPC * D], BF16),
        din("bv", [HPC * D], F32),
        din("wcp", [HPC * D, C], BF16),
        din("bcp", [C], F32),
        din("wfc", [C, FF], BF16),
        din("bfc", [FF], F32),
        din("wmp", [FF, C], BF16),
        din("bmp", [C], F32),
        din("mask", [128, 896], BF16),
        nc.dram_tensor("out", [TS, C], F32, kind="ExternalOutput").ap(),
    )
    with tile.TileContext(nc) as tc, ExitStack() as ctx:
        pools = _Pools(ctx, tc)
        if loop_n > 1:
            with tc.For_i(0, loop_n, 1):
                _body(pools, nc, tc, io, timing=True,
                      skip_att=skip_att, skip_mlp=skip_mlp)
        else:
            _body(pools, nc, tc, io, timing=timing,
                  skip_att=skip_att, skip_mlp=skip_mlp)
    nc.finalize()
    _BUILT[key] = nc
    return nc


def make_in_maps(inputs):
    """Host-side sharding: full inputs dict -> per-core in_maps."""
    f32 = np.float32
    bf = ml_dtypes.bfloat16
    x = np.asarray(inputs["x"], f32)
    ln1_g = np.asarray(inputs["ln1_g"], f32)
    ln1_b = np.asarray(inputs["ln1_b"], f32)
    W_attn = np.asarray(inputs["W_attn"], f32)
    b_attn = np.asarray(inputs["b_attn"], f32)
    W_cproj = np.asarray(inputs["W_cproj"], f32)
    b_cproj = np.asarray(inputs["b_cproj"], f32)
    ln2_g = np.asarray(inputs["ln2_g"], f32)
    ln2_b = np.asarray(inputs["ln2_b"], f32)
    W_fc = np.asarray(inputs["W_fc"], f32)
    b_fc = np.asarray(inputs["b_fc"], f32)
    W_mproj = np.asarray(inputs["W_mproj"], f32)
    b_mproj = np.asarray(inputs["b_mproj"], f32)

    Wa = ln1_g[:, None] * W_attn
    ba = b_attn + ln1_b @ W_attn
    Wf = ln2_g[:, None] * W_fc
    bf_ = b_fc + ln2_b @ W_fc

    p = np.arange(128)[:, None]
    c = np.arange(896)[None, :]
    mask = (c >= p + 384).astype(bf)

    maps = []
    for core in range(NCORES):
        b, s = core // 4, core % 4
        q0 = 192 * s
        zpad = np.zeros((C, 64), f32)
        # [Q0 Q1 | K0 K1 | Q2 pad | K2 pad]
        wqk = np.concatenate([
            Wa[:, q0:q0 + 128], Wa[:, 768 + q0:768 + q0 + 128],
            Wa[:, q0 + 128:q0 + 192], zpad,
            Wa[:, 768 + q0 + 128:768 + q0 + 192], zpad], axis=1)
        bqk = np.concatenate([
            ba[q0:q0 + 128], ba[768 + q0:768 + q0 + 128],
            ba[q0 + 128:q0 + 192], np.zeros(64, f32),
            ba[768 + q0 + 128:768 + q0 + 192], np.zeros(64, f32)])
        maps.append({
            "x": np.ascontiguousarray(x[b]),
            "xs": np.ascontiguousarray(x[b, TS * s:TS * (s + 1)] + b_cproj),
            "wqk": np.ascontiguousarray(wqk.astype(bf)),
            "bqk": np.ascontiguousarray(bqk),
            "wv": np.ascontiguousarray(Wa[:, 1536 + q0:1536 + q0 + 192].astype(bf)),
            "bv": np.ascontiguousarray(ba[1536 + q0:1536 + q0 + 192]),
            "wcp": np.ascontiguousarray(W_cproj[q0:q0 + 192, :].astype(bf)),
            "bcp": b_cproj,
            "wfc": np.ascontiguousarray(Wf.astype(bf)),
            "bfc": bf_,
            "wmp": np.ascontiguousarray(W_mproj.astype(bf)),
            "bmp": b_mproj,
            "mask": mask,
        })
    return maps


def _get_runner():
    """Persistent jitted 8-core dispatch (replicates bass2jax.run_bass_via_pjrt
    but keeps the compiled executable so repeated kernel() calls are cheap)."""
    if "runner" in _BUILT:
        return _BUILT["runner"]
    import jax
    from jax.sharding import Mesh, PartitionSpec, NamedSharding
    from jax.experimental.shard_map import shard_map
    from concourse import bass2jax

    nc = build()
    bass2jax.install_neuronx_cc_hook()
    part_name = nc.partition_id_tensor.name if nc.partition_id_tensor else None
    in_names, out_names, out_avals, zero_shapes = [], [], [], []
    for alloc in nc.m.functions[0].allocations:
        if not isinstance(alloc, mybir.MemoryLocationSet):
            continue
        name = alloc.memorylocations[0].name
        if alloc.kind == "ExternalInput":
            if name != part_name:
                in_names.append(name)
        elif alloc.kind == "ExternalOutput":
            out_names.append(name)
            shape = tuple(alloc.tensor_shape)
            dtype = mybir.dt.np(alloc.dtype)
            out_avals.append(jax.core.ShapedArray(shape, dtype))
            zero_shapes.append((shape, dtype))
    n_params, n_outs = len(in_names), len(out_names)
    all_names = in_names + out_names + ([part_name] if part_name else [])

    def _fn(*args):
        args = list(args)
        if part_name is not None:
            args.append(bass2jax.partition_id_tensor())
        return tuple(bass2jax.bass_exec(out_avals, all_names, out_names, nc, {},
                                        True, True, *args))

    devices = jax.devices()[:NCORES]
    mesh = Mesh(np.asarray(devices), ("core",))
    sharded = jax.jit(
        shard_map(_fn, mesh=mesh,
                  in_specs=(PartitionSpec("core"),) * (n_params + n_outs),
                  out_specs=(PartitionSpec("core"),) * n_outs, check_rep=False),
        donate_argnums=tuple(range(n_params, n_params + n_outs)), keep_unused=True)
    sh = NamedSharding(mesh, PartitionSpec("core"))

    def run(maps):
        concat_in = [jax.device_put(np.concatenate(
            [np.asarray(maps[c][nm]) for c in range(NCORES)], axis=0), sh)
            for nm in in_names]
        zeros = [jax.device_put(
            np.zeros((NCORES * shp[0], *shp[1:]), dt), sh)
            for shp, dt in zero_shapes]
        outs = sharded(*concat_in, *zeros)
        i = out_names.index("out")
        return np.asarray(outs[i]).reshape(NCORES, TS, C)

    _BUILT["runner"] = run
    return run


def kernel(**inputs):
    maps = make_in_maps(inputs)
    run = _get_runner()
    per_core = run(maps)
    out = np.empty((B, T, C), np.float32)
    for core in range(NCORES):
        b, s = core // 4, core % 4
        out[b, TS * s:TS * (s + 1)] = per_core[core]
    return out
